# revision 1
# baseline (speedup 1.0000x reference)
"""Masked-gather L1 loss on 8 Trainium2 NeuronCores.

Strategy (data-parallel over batch, 4 batches per core):
  - Stream pred[b] ([128 c, 25600 hw] f32, 13.1 MB) into SBUF, double-buffered.
  - GPSIMD ap_gather pulls the 1024 indexed columns out of SBUF:
    mid[c, k] = pred[c, idx_k]  (same index for every channel).
  - DVE: diff = mid - target;  ACT: |diff| in place.
  - PE: ones[128,1]^T @ |diff| -> per-k column sums in PSUM.
  - DVE: multiply by mask[b], reduce -> per-batch scalar.
  - Each core returns [sum_b sum_ck m_k|t-p|, sum_b sum_k m_k]; host combines
    the 8 partial pairs and applies total / (mask_sum * C + eps).
"""

import sys

sys.path.insert(0, "/opt/trn_rl_repo")

import numpy as np

B, C, H, W = 32, 128, 160, 160
K = 1024
HW = H * W
N_CORES = 8
BPC = B // N_CORES  # batches per core
EPS = 1e-5

_CACHE = {}


def _build(repeats=1, dma_split=1):
    from contextlib import ExitStack

    from concourse import bacc, mybir, tile

    f32 = mybir.dt.float32
    i16 = mybir.dt.int16

    nc = bacc.Bacc(
        "TRN2",
        target_bir_lowering=False,
        debug=False,
        num_devices=N_CORES,
        dynamic_dma_scratch_size=4096,
    )

    pred_d = nc.dram_tensor("pred", [BPC, C, HW], f32, kind="ExternalInput")
    target_d = nc.dram_tensor("target", [BPC, C, K], f32, kind="ExternalInput")
    idx_d = nc.dram_tensor("idx", [C, BPC * (K // 16)], i16, kind="ExternalInput")
    mask_d = nc.dram_tensor("mask", [BPC, K], f32, kind="ExternalInput")
    out_d = nc.dram_tensor("out", [1, 2], f32, kind="ExternalOutput")

    IDXW = K // 16  # 64 idx slots per partition per batch

    with tile.TileContext(nc) as tc, ExitStack() as ctx:
        pred_pool = ctx.enter_context(tc.tile_pool(name="pred", bufs=2))
        mid_pool = ctx.enter_context(tc.tile_pool(name="mid", bufs=2))
        tgt_pool = ctx.enter_context(tc.tile_pool(name="tgt", bufs=1))
        msk_pool = ctx.enter_context(tc.tile_pool(name="msk", bufs=1))
        singles = ctx.enter_context(tc.tile_pool(name="singles", bufs=1))
        psum = ctx.enter_context(tc.tile_pool(name="psum", bufs=2, space="PSUM"))

        idx_t = singles.tile([C, BPC * IDXW], i16)
        nc.sync.dma_start(idx_t[:], idx_d.ap()[:])
        ones_t = singles.tile([C, 1], f32)
        nc.vector.memset(ones_t[:], 1.0)
        acc_t = singles.tile([1, 2 * BPC], f32)
        nc.vector.memset(acc_t[:], 0.0)
        tmp_t = singles.tile([1, 2], f32)
        fin_t = singles.tile([1, 2], f32)

        for b in [b for _ in range(repeats) for b in range(BPC)]:
            pt = pred_pool.tile([C, HW], f32)
            if dma_split == 1:
                nc.sync.dma_start(pt[:], pred_d.ap()[b])
            else:
                CH = HW // dma_split
                for s in range(dma_split):
                    eng = nc.sync if s % 2 == 0 else nc.scalar
                    eng.dma_start(
                        pt[:, s * CH : (s + 1) * CH],
                        pred_d.ap()[b, :, s * CH : (s + 1) * CH],
                    )
            tt = tgt_pool.tile([C, K], f32)
            nc.sync.dma_start(tt[:], target_d.ap()[b])
            mt = msk_pool.tile([1, K], f32)
            nc.sync.dma_start(mt[:], mask_d.ap()[b : b + 1])

            gt = mid_pool.tile([C, K], f32)
            nc.gpsimd.ap_gather(
                gt[:],
                pt[:],
                idx_t[:, b * IDXW : (b + 1) * IDXW],
                channels=C,
                num_elems=HW,
                d=1,
                num_idxs=K,
            )
            nc.vector.tensor_tensor(
                gt[:], gt[:], tt[:], op=mybir.AluOpType.subtract
            )
            nc.scalar.activation(gt[:], gt[:], mybir.ActivationFunctionType.Abs)

            ps = psum.tile([1, K], f32)
            nc.tensor.matmul(ps[:, 0:512], ones_t[:], gt[:, 0:512])
            nc.tensor.matmul(ps[:, 512:1024], ones_t[:], gt[:, 512:1024])
            nc.vector.tensor_tensor(ps[:], ps[:], mt[:], op=mybir.AluOpType.mult)
            nc.vector.tensor_reduce(
                tmp_t[:, 0:1],
                ps[:],
                axis=mybir.AxisListType.X,
                op=mybir.AluOpType.add,
            )
            nc.vector.tensor_tensor(
                acc_t[:, b : b + 1],
                acc_t[:, b : b + 1],
                tmp_t[:, 0:1],
                op=mybir.AluOpType.add,
            )
            nc.vector.tensor_reduce(
                tmp_t[:, 1:2],
                mt[:],
                axis=mybir.AxisListType.X,
                op=mybir.AluOpType.add,
            )
            nc.vector.tensor_tensor(
                acc_t[:, BPC + b : BPC + b + 1],
                acc_t[:, BPC + b : BPC + b + 1],
                tmp_t[:, 1:2],
                op=mybir.AluOpType.add,
            )

        nc.vector.tensor_reduce(
            fin_t[:, 0:1],
            acc_t[:, 0:BPC],
            axis=mybir.AxisListType.X,
            op=mybir.AluOpType.add,
        )
        nc.vector.tensor_reduce(
            fin_t[:, 1:2],
            acc_t[:, BPC : 2 * BPC],
            axis=mybir.AxisListType.X,
            op=mybir.AluOpType.add,
        )
        nc.sync.dma_start(out_d.ap()[:], fin_t[:])

    nc.compile()
    return nc


def _get_nc(repeats=1, dma_split=1):
    key = ("nc", repeats, dma_split)
    if key not in _CACHE:
        _CACHE[key] = _build(repeats, dma_split)
    return _CACHE[key]


def make_in_maps(pred, target, indices, mask):
    pred = np.ascontiguousarray(np.asarray(pred), dtype=np.float32)
    target = np.ascontiguousarray(np.asarray(target), dtype=np.float32)
    indices = np.asarray(indices)
    mask = np.ascontiguousarray(np.asarray(mask), dtype=np.float32)

    predf = pred.reshape(B, C, HW)
    # ap_gather index layout: within each 16-partition group, index j lives at
    # (partition j % 16, slot j // 16); replicated across the 8 groups.
    idxw = indices.reshape(B, K // 16, 16).transpose(0, 2, 1)  # [B, 16, 64]
    idxt = np.tile(idxw, (1, C // 16, 1)).astype(np.int16)  # [B, 128, 64]

    in_maps = []
    for core in range(N_CORES):
        sl = slice(core * BPC, (core + 1) * BPC)
        idx_core = np.ascontiguousarray(
            idxt[sl].transpose(1, 0, 2)
        ).reshape(C, BPC * (K // 16))
        in_maps.append(
            {
                "pred": np.ascontiguousarray(predf[sl]),
                "target": target[sl],
                "idx": idx_core,
                "mask": mask[sl],
            }
        )
    return in_maps


def run(pred, target, indices, mask, trace=False, **rk_kwargs):
    from concourse.bass_utils import run_bass_kernel_spmd

    nc = _get_nc()
    in_maps = make_in_maps(pred, target, indices, mask)
    res = run_bass_kernel_spmd(
        nc, in_maps, list(range(N_CORES)), trace=trace, **rk_kwargs
    )
    parts = np.stack([r["out"][0] for r in res.results])  # [8, 2]
    total = float(parts[:, 0].sum())
    mask_sum = float(parts[:, 1].sum())
    out = np.float32(total / (mask_sum * C + EPS))
    return out, res


def kernel(pred, target, indices, mask):
    out, _ = run(pred, target, indices, mask)
    return out



# revision 3
# speedup vs baseline: 1.1806x; 1.1806x over previous
"""Masked-gather L1 loss on 8 Trainium2 NeuronCores — indirect-DMA version.

Only 1024 of 25600 spatial positions per batch are needed (4%). Ship pred
host-transposed to [HW, C] so each needed position is one contiguous 512 B
row, then SWDGE dma_gather pulls exactly the 1024 rows per batch straight
from HBM into SBUF: 2 MB/core instead of 52 MB/core (~13x less HBM traffic
than streaming pred). Each batch's gather runs on its own SWDGE queue
(queue_num=b, num_swdge_queues=4) so descriptor generation parallelizes
across the 4 GPSIMD Q7 core pairs — measured 2.8x faster than all gathers
on queue 0. Target is shipped host-pre-arranged to the gather's
[k%128, k//128, c] SBUF layout and streamed per batch (512 KB tiles,
alternating the two HWDGE rings).

Per core (4 batches):
  - dma_gather: gt[p, g, c] = pred_t[b, idx[b, g*128+p], c]   [128, 8, 128]
  - DVE: diff = gt - tgt_tile(b)
  - DVE: st[p, b*8+g] = sum_c |diff|   (tensor_reduce with fused abs)
  - After all batches: st *= mask_t; wsum/msum reduce + ones^T matmul
    -> [1, 2] per-core output.
Host combines the 8 partial pairs: total / (mask_sum * C + eps).
"""

import sys

sys.path.insert(0, "/opt/trn_rl_repo")

import numpy as np

B, C, H, W = 32, 128, 160, 160
K = 1024
HW = H * W
N_CORES = 8
BPC = B // N_CORES  # batches per core
G = K // 128  # gather groups per batch (8)
IDXW = K // 16  # idx slots per partition per batch (64)
EPS = 1e-5

_CACHE = {}


def _build(repeats=1, hw_loop=False):
    from contextlib import ExitStack

    from concourse import bacc, mybir, tile

    f32 = mybir.dt.float32
    i16 = mybir.dt.int16

    nc = bacc.Bacc(
        "TRN2",
        target_bir_lowering=False,
        debug=False,
        num_devices=N_CORES,
        dynamic_dma_scratch_size=16384,
        num_swdge_queues=4,
    )

    pred_d = nc.dram_tensor("pred", [BPC, HW, C], f32, kind="ExternalInput")
    tgt_d = nc.dram_tensor("target", [C, BPC * G, C], f32, kind="ExternalInput")
    idx_d = nc.dram_tensor("idx", [C, BPC * IDXW], i16, kind="ExternalInput")
    msk_d = nc.dram_tensor("mask", [C, BPC * G], f32, kind="ExternalInput")
    out_d = nc.dram_tensor("out", [1, 2], f32, kind="ExternalOutput")

    with tile.TileContext(nc) as tc, ExitStack() as ctx:
        gat_pool = ctx.enter_context(tc.tile_pool(name="gat", bufs=4))
        tgt_pool = ctx.enter_context(tc.tile_pool(name="tgt", bufs=4))
        singles = ctx.enter_context(tc.tile_pool(name="singles", bufs=1))
        psum = ctx.enter_context(tc.tile_pool(name="psum", bufs=1, space="PSUM"))

        idx_t = singles.tile([C, BPC * IDXW], i16)
        nc.sync.dma_start(idx_t[:], idx_d.ap()[:])
        msk_t = singles.tile([C, BPC * G], f32)
        nc.sync.dma_start(msk_t[:], msk_d.ap()[:])
        ones_t = singles.tile([C, 1], f32)
        nc.vector.memset(ones_t[:], 1.0)
        st_t = singles.tile([C, BPC * G], f32)
        pair_t = singles.tile([C, 2], f32)
        fin_t = singles.tile([1, 2], f32)

        def batch_pass(b):
            tt = tgt_pool.tile([C, G, C], f32)
            eng = nc.sync if b % 2 == 0 else nc.scalar
            eng.dma_start(tt[:], tgt_d.ap()[:, b * G : (b + 1) * G])
            gt = gat_pool.tile([C, G, C], f32)
            nc.gpsimd.dma_gather(
                gt[:],
                pred_d.ap()[b],
                idx_t[:, b * IDXW : (b + 1) * IDXW],
                num_idxs=K,
                num_idxs_reg=K,
                elem_size=C,
                queue_num=b,
            )
            nc.vector.tensor_tensor(
                gt[:], gt[:], tt[:], op=mybir.AluOpType.subtract
            )
            nc.vector.tensor_reduce(
                st_t[:, b * G : (b + 1) * G],
                gt[:],
                axis=mybir.AxisListType.X,
                op=mybir.AluOpType.add,
                apply_absolute_value=True,
            )

        if hw_loop and repeats > 1:
            # 8 unrolled passes per HW-loop iteration to amortize the
            # all-engine back-edge sync; repeats must be a multiple of 8.
            assert repeats % 8 == 0
            with tc.For_i(0, repeats // 8) as _:
                for _u in range(8):
                    for b in range(BPC):
                        batch_pass(b)
        else:
            for b in [b for _ in range(repeats) for b in range(BPC)]:
                batch_pass(b)

        nc.vector.tensor_tensor(
            st_t[:], st_t[:], msk_t[:], op=mybir.AluOpType.mult
        )
        nc.vector.tensor_reduce(
            pair_t[:, 0:1],
            st_t[:],
            axis=mybir.AxisListType.X,
            op=mybir.AluOpType.add,
        )
        nc.vector.tensor_reduce(
            pair_t[:, 1:2],
            msk_t[:],
            axis=mybir.AxisListType.X,
            op=mybir.AluOpType.add,
        )
        ps = psum.tile([1, 2], f32)
        nc.tensor.matmul(ps[:], ones_t[:], pair_t[:])
        nc.scalar.copy(fin_t[:], ps[:])
        nc.sync.dma_start(out_d.ap()[:], fin_t[:])

    nc.compile()
    return nc


def _get_nc(repeats=1, hw_loop=False):
    key = ("nc", repeats, hw_loop)
    if key not in _CACHE:
        _CACHE[key] = _build(repeats, hw_loop)
    return _CACHE[key]


def make_in_maps(pred, target, indices, mask):
    pred = np.ascontiguousarray(np.asarray(pred), dtype=np.float32)
    target = np.ascontiguousarray(np.asarray(target), dtype=np.float32)
    indices = np.asarray(indices)
    mask = np.ascontiguousarray(np.asarray(mask), dtype=np.float32)

    predf = pred.reshape(B, C, HW)
    # dma_gather index layout: index j lives at (partition j % 16,
    # slot j // 16), replicated across the 8 16-partition groups.
    idxw = indices.reshape(B, IDXW, 16).transpose(0, 2, 1)  # [B, 16, 64]
    idxt = np.tile(idxw, (1, C // 16, 1)).astype(np.int16)  # [B, 128, 64]

    in_maps = []
    for core in range(N_CORES):
        sl = slice(core * BPC, (core + 1) * BPC)
        # pred -> [BPC, HW, C]: one contiguous 512B row per spatial position
        pred_core = np.ascontiguousarray(predf[sl].transpose(0, 2, 1))
        # target -> [p, b, g, c] = target[b, c, g*128+p]
        tgt_core = np.ascontiguousarray(
            target[sl].reshape(BPC, C, G, 128).transpose(3, 0, 2, 1)
        ).reshape(C, BPC * G, C)
        # mask -> [p, b*G+g] = mask[b, g*128+p]
        msk_core = np.ascontiguousarray(
            mask[sl].reshape(BPC, G, 128).transpose(2, 0, 1)
        ).reshape(C, BPC * G)
        idx_core = np.ascontiguousarray(
            idxt[sl].transpose(1, 0, 2)
        ).reshape(C, BPC * IDXW)
        in_maps.append(
            {
                "pred": pred_core,
                "target": tgt_core,
                "idx": idx_core,
                "mask": msk_core,
            }
        )
    return in_maps


def combine(results):
    parts = np.stack([r["out"][0] for r in results])  # [8, 2]
    total = float(parts[:, 0].sum())
    mask_sum = float(parts[:, 1].sum())
    return np.float32(total / (mask_sum * C + EPS))


def run(pred, target, indices, mask, repeats=1, **rk_kwargs):
    from concourse.bass_utils import run_bass_kernel_spmd

    nc = _get_nc(repeats)
    in_maps = make_in_maps(pred, target, indices, mask)
    res = run_bass_kernel_spmd(
        nc, in_maps, list(range(N_CORES)), **rk_kwargs
    )
    return combine(res.results), res


def kernel(pred, target, indices, mask):
    out, _ = run(pred, target, indices, mask)
    return out


# revision 5
# speedup vs baseline: 1.5251x; 1.2919x over previous
"""Masked-gather L1 loss on 8 Trainium2 NeuronCores — indirect-DMA version.

Only 1024 of 25600 spatial positions per batch are needed (4%). Ship pred
host-transposed to [HW, C] so each needed position is one contiguous 512 B
row, then SWDGE dma_gather pulls exactly the 1024 rows per batch straight
from HBM into SBUF: 2 MB/core instead of 52 MB/core (~13x less HBM
traffic). Each batch's gather runs on its own SWDGE queue (queue_num=b,
num_swdge_queues=4) so descriptor generation parallelizes across the 4
GPSIMD Q7 core pairs (2.8x over one queue). Target is shipped
host-pre-arranged to the gather's [k%128, k//128, c] SBUF layout and
streamed per batch (512 KB tiles, alternating the two HWDGE rings).
8-deep tile pools give two full passes of buffering so the next pass's
gather descgen overlaps this pass's DVE work (15.2 vs 19.5 us/pass).

Per core (4 batches):
  - dma_gather: gt[p, g, c] = pred_t[b, idx[b, g*128+p], c]   [128, 8, 128]
  - DVE: diff = gt - tgt_tile(b)
  - DVE: st[p, b*8+g] = sum_c |diff|   (tensor_reduce with fused abs)
  - After all batches: st *= mask_t; reduce -> [128, 2] per-core partials
    (numerator, mask-sum per partition); no PE/ACT tail.
Host sums the 8x128 partial pairs: total / (mask_sum * C + eps).
"""

import sys

sys.path.insert(0, "/opt/trn_rl_repo")

import numpy as np

B, C, H, W = 32, 128, 160, 160
K = 1024
HW = H * W
N_CORES = 8
BPC = B // N_CORES  # batches per core
G = K // 128  # gather groups per batch (8)
IDXW = K // 16  # idx slots per partition per batch (64)
EPS = 1e-5

_CACHE = {}


def _build(repeats=1, hw_loop=False):
    from contextlib import ExitStack

    from concourse import bacc, mybir, tile

    f32 = mybir.dt.float32
    i16 = mybir.dt.int16

    nc = bacc.Bacc(
        "TRN2",
        target_bir_lowering=False,
        debug=False,
        num_devices=N_CORES,
        dynamic_dma_scratch_size=16384,
        num_swdge_queues=4,
    )

    pred_d = nc.dram_tensor("pred", [BPC, HW, C], f32, kind="ExternalInput")
    tgt_d = nc.dram_tensor("target", [C, BPC * G, C], f32, kind="ExternalInput")
    idx_d = nc.dram_tensor("idx", [C, BPC * IDXW], i16, kind="ExternalInput")
    msk_d = nc.dram_tensor("mask", [C, BPC * G], f32, kind="ExternalInput")
    out_d = nc.dram_tensor("out", [C, 2], f32, kind="ExternalOutput")

    with tile.TileContext(nc) as tc, ExitStack() as ctx:
        gat_pool = ctx.enter_context(tc.tile_pool(name="gat", bufs=8))
        tgt_pool = ctx.enter_context(tc.tile_pool(name="tgt", bufs=8))
        singles = ctx.enter_context(tc.tile_pool(name="singles", bufs=1))

        idx_t = singles.tile([C, BPC * IDXW], i16)
        nc.sync.dma_start(idx_t[:], idx_d.ap()[:])
        msk_t = singles.tile([C, BPC * G], f32)
        nc.sync.dma_start(msk_t[:], msk_d.ap()[:])
        st_t = singles.tile([C, BPC * G], f32)
        pair_t = singles.tile([C, 2], f32)

        def batch_pass(b):
            tt = tgt_pool.tile([C, G, C], f32)
            eng = nc.sync if b % 2 == 0 else nc.scalar
            eng.dma_start(tt[:], tgt_d.ap()[:, b * G : (b + 1) * G])
            gt = gat_pool.tile([C, G, C], f32)
            nc.gpsimd.dma_gather(
                gt[:],
                pred_d.ap()[b],
                idx_t[:, b * IDXW : (b + 1) * IDXW],
                num_idxs=K,
                num_idxs_reg=K,
                elem_size=C,
                queue_num=b,
            )
            nc.vector.tensor_tensor(
                gt[:], gt[:], tt[:], op=mybir.AluOpType.subtract
            )
            nc.vector.tensor_reduce(
                st_t[:, b * G : (b + 1) * G],
                gt[:],
                axis=mybir.AxisListType.X,
                op=mybir.AluOpType.add,
                apply_absolute_value=True,
            )

        if hw_loop and repeats > 1:
            # 8 unrolled passes per HW-loop iteration to amortize the
            # all-engine back-edge sync; repeats must be a multiple of 8.
            assert repeats % 8 == 0
            with tc.For_i(0, repeats // 8) as _:
                for _u in range(8):
                    for b in range(BPC):
                        batch_pass(b)
        else:
            for b in [b for _ in range(repeats) for b in range(BPC)]:
                batch_pass(b)

        nc.vector.tensor_tensor(
            st_t[:], st_t[:], msk_t[:], op=mybir.AluOpType.mult
        )
        nc.vector.tensor_reduce(
            pair_t[:, 0:1],
            st_t[:],
            axis=mybir.AxisListType.X,
            op=mybir.AluOpType.add,
        )
        nc.vector.tensor_reduce(
            pair_t[:, 1:2],
            msk_t[:],
            axis=mybir.AxisListType.X,
            op=mybir.AluOpType.add,
        )
        # ship [128, 2] per-partition partials; host sums 128 rows x 8 cores
        nc.sync.dma_start(out_d.ap()[:], pair_t[:])

    nc.compile()
    return nc


def _get_nc(repeats=1, hw_loop=False):
    key = ("nc", repeats, hw_loop)
    if key not in _CACHE:
        _CACHE[key] = _build(repeats, hw_loop)
    return _CACHE[key]


def make_in_maps(pred, target, indices, mask):
    pred = np.ascontiguousarray(np.asarray(pred), dtype=np.float32)
    target = np.ascontiguousarray(np.asarray(target), dtype=np.float32)
    indices = np.asarray(indices)
    mask = np.ascontiguousarray(np.asarray(mask), dtype=np.float32)

    predf = pred.reshape(B, C, HW)
    # dma_gather index layout: index j lives at (partition j % 16,
    # slot j // 16), replicated across the 8 16-partition groups.
    idxw = indices.reshape(B, IDXW, 16).transpose(0, 2, 1)  # [B, 16, 64]
    idxt = np.tile(idxw, (1, C // 16, 1)).astype(np.int16)  # [B, 128, 64]

    in_maps = []
    for core in range(N_CORES):
        sl = slice(core * BPC, (core + 1) * BPC)
        # pred -> [BPC, HW, C]: one contiguous 512B row per spatial position
        pred_core = np.ascontiguousarray(predf[sl].transpose(0, 2, 1))
        # target -> [p, b, g, c] = target[b, c, g*128+p]
        tgt_core = np.ascontiguousarray(
            target[sl].reshape(BPC, C, G, 128).transpose(3, 0, 2, 1)
        ).reshape(C, BPC * G, C)
        # mask -> [p, b*G+g] = mask[b, g*128+p]
        msk_core = np.ascontiguousarray(
            mask[sl].reshape(BPC, G, 128).transpose(2, 0, 1)
        ).reshape(C, BPC * G)
        idx_core = np.ascontiguousarray(
            idxt[sl].transpose(1, 0, 2)
        ).reshape(C, BPC * IDXW)
        in_maps.append(
            {
                "pred": pred_core,
                "target": tgt_core,
                "idx": idx_core,
                "mask": msk_core,
            }
        )
    return in_maps


def combine(results):
    parts = np.stack([r["out"] for r in results])  # [8, 128, 2]
    total = float(parts[:, :, 0].sum(dtype=np.float64))
    mask_sum = float(parts[:, :, 1].sum(dtype=np.float64))
    return np.float32(total / (mask_sum * C + EPS))


def run(pred, target, indices, mask, repeats=1, **rk_kwargs):
    from concourse.bass_utils import run_bass_kernel_spmd

    nc = _get_nc(repeats)
    in_maps = make_in_maps(pred, target, indices, mask)
    res = run_bass_kernel_spmd(
        nc, in_maps, list(range(N_CORES)), **rk_kwargs
    )
    return combine(res.results), res


def kernel(pred, target, indices, mask):
    out, _ = run(pred, target, indices, mask)
    return out
